# revision 27
# baseline (speedup 1.0000x reference)
"""Cross-attention block on 8 Trainium2 NeuronCores (v2).

Computes, per batch b:
    xn = LN(x); cn = LN(cond)
    q = xn @ Wq; k = cn @ Wk; v = cn @ Wv   (8 heads x 64)
    out = softmax(q k^T / sqrt(64)) v
    y  = LN(out @ Wo + bo + x)

Sharding: 8 cores = 4 batches x 2 query-row halves (data parallel over
(batch, query-block)), no collectives.

v2 structure (vs the v1 baseline):
  * Scores are computed as row-tiled matmul PAIRS: head A's K-chunk loads
    into PE rows 0:63 (tile_position (0,0)) and head B's into rows 64:127
    ((64,0)); the two 512-col streams co-issue, halving score time.
  * PV uses fp8(e4m3) DoubleRow: the stationary holds 2 keys per cell
    ([128, 2, 65] V-chunks), the moving operand is exp(scores) written by
    ACT directly in fp8 as [128, 2, 512] double-chunks.  256 keys per
    matmul -> half the PV streams.  exp carries a free bias of -1.5 so
    values stay inside e4m3 range; the shift cancels in softmax.
  * The softmax denominator stays fused as a 65th ones-column of V.
  * All LN rstd values use exp(-0.5*ln(var+eps)) so the single ACT table
    set (natural_log_exp) serves the whole kernel - no table switches.
  * LN centering/scaling runs on DVE (tensor_scalar), not ACT: ACT is the
    critical engine (exp stream ~128us) and does nothing else during
    attention.
  * Matmuls are grouped by PE tiling mode (64-row score groups vs 128-row
    PV/projection groups) to amortize the mode-switch drain.
  * Projections / LN transposes / Wo / residual loads are emitted as
    background thunks interleaved between attention groups so the Tile
    scheduler can fill PE slack under the ACT-bound exp stream and keep
    the PE HAM-warm.
"""

import functools

import numpy as np

B, N, M = 4, 2048, 2048
DQ, DC = 512, 768
H, DH = 8, 64
INNER = H * DH  # 512
P = 128
NQ = N // 2  # query rows per core
EPS = 1e-5
N_CORES = 8

FC_X = DQ // P  # 4 feature chunks of x
FC_C = DC // P  # 6 feature chunks of cond
IC = INNER // P  # 4 inner chunks (= head pairs)
TK = M // P  # 16 key-token chunks
TK2 = TK // 2  # 8 double (256-key) chunks
NT = NQ // 512  # 2 query column tiles
VP = 80  # padded fp8 V row stride (step%16==0)
SCALE = float(DH) ** -0.5
ESHIFT = -1.0  # exp range shift for fp8; cancels in softmax

FP8_PV = True


def _emit(tc, io):
    import contextlib
    import math

    import concourse.bass as bass
    import concourse.mybir as mybir

    nc = tc.nc
    f32 = mybir.dt.float32
    bf16 = mybir.dt.bfloat16
    f8 = mybir.dt.float8e4
    AF = mybir.ActivationFunctionType
    OP = mybir.AluOpType
    PM = mybir.MatmulPerfMode

    pdt = f8 if FP8_PV else bf16

    ctx = contextlib.ExitStack()
    with ctx:
        singles = ctx.enter_context(tc.tile_pool(name="singles", bufs=1))
        wst = ctx.enter_context(tc.tile_pool(name="wst", bufs=1))
        work = ctx.enter_context(tc.tile_pool(name="work", bufs=3))
        stat = ctx.enter_context(tc.tile_pool(name="stat", bufs=4))
        cenp = ctx.enter_context(tc.tile_pool(name="cenp", bufs=5))
        pp = ctx.enter_context(tc.tile_pool(name="pp", bufs=4))
        xresp = ctx.enter_context(tc.tile_pool(name="xresp", bufs=1))
        ps = ctx.enter_context(tc.tile_pool(name="ps", bufs=1, space="PSUM"))

        # ---- constants -------------------------------------------------
        from concourse.masks import make_identity

        ident = singles.tile([P, P], bf16, name="ident")
        make_identity(nc, ident)
        eps_t = singles.tile([P, 1], f32, name="eps_t")
        nc.vector.memset(eps_t, EPS)
        esh_t = singles.tile([P, 1], f32, name="esh_t")
        nc.vector.memset(esh_t, ESHIFT)

        def bcast_load(vec_ap, width, name):
            t = singles.tile([P, width], f32, name=name)
            bc = bass.AP(
                tensor=vec_ap.tensor,
                offset=vec_ap.offset,
                ap=[[0, P]] + [list(a) for a in vec_ap.ap],
            )
            nc.gpsimd.dma_start(out=t, in_=bc)
            return t

        def strip_load(vec_ap, chunks, name):
            t = singles.tile([P, chunks], f32, name=name)
            nc.sync.dma_start(out=t, in_=vec_ap.rearrange("(c p) -> p c", p=P))
            return t

        gx = strip_load(io["lnx_g"], FC_X, "gx")
        bx = strip_load(io["lnx_b"], FC_X, "bx")
        gc = strip_load(io["lnc_g"], FC_C, "gc")
        bc_ = strip_load(io["lnc_b"], FC_C, "bc")
        gf_bc = bcast_load(io["lnf_g"], DQ, "gf_bc")
        bf_bc = bcast_load(io["lnf_b"], DQ, "bf_bc")
        bo_bc = bcast_load(io["bo"], DQ, "bo_bc")

        # ---- weights: fp32 HBM -> bf16 SBUF, contraction on partitions --
        def load_weight(w_ap, din, name):
            kc = din // P
            stage = wst.tile([P, kc, INNER], f32, tag="wstage", bufs=1,
                             name=f"{name}_st")
            nc.sync.dma_start(
                out=stage, in_=w_ap.rearrange("(ko p) i -> p ko i", p=P)
            )
            wb = singles.tile([P, kc, INNER], bf16, name=name)
            nc.scalar.copy(out=wb, in_=stage)
            return wb

        def load_wo():
            wo_st = wst.tile([DH, H, DQ], f32, tag="wostage", name="wo_st")
            nc.sync.dma_start(
                out=wo_st, in_=io["Wo"].rearrange("(h p) d -> p h d", p=DH)
            )
            wo_b = singles.tile([DH, H, DQ], bf16, name="wo_b")
            nc.vector.tensor_copy(out=wo_b, in_=wo_st)
            return wo_b

        # ---- persistent activations ------------------------------------
        xnT = singles.tile([P, FC_X, NQ], bf16, name="xnT")
        cnT = singles.tile([P, FC_C, M], bf16, name="cnT")
        QT = singles.tile([P, IC, NQ], bf16, name="QT")
        KT = singles.tile([P, IC, M], bf16, name="KT")
        if FP8_PV:
            # V8[:, g, j, h, 0:64] = v for key chunk 2g+j; [..., 64] = 1
            # (fused softmax denominator); cols 65:80 pad for step%16==0.
            V8 = singles.tile([P, TK2, 2, H, VP], f8, name="V8")
            nc.vector.memset(V8, 1.0)
        else:
            V8 = singles.tile([P, TK, H, DH + 1], bf16, name="V8")
            nc.vector.memset(V8, 1.0)
        OT = singles.tile([DH, H, NQ], bf16, name="OT")

        # ---- LN + PE-transpose into feature-major layout ---------------
        def ln_dma(src, width, tg, state):
            """DMA 4 token-chunks + stats (no ACT work: the ACT queue is
            strict FIFO and must never wait on a fresh DMA mid-attention)."""
            fmax = math.gcd(512, width)
            nsub = width // fmax
            xs, mvs = [], []
            for tl in range(4):
                x_t = work.tile([P, width], f32, tag="xin", bufs=4, name="x_t")
                nc.sync.dma_start(out=x_t, in_=src[:, tg * 4 + tl])
                if nsub == 1:
                    stats = stat.tile([P, 6], f32, tag="bns", bufs=6, name="st6")
                    nc.vector.bn_stats(out=stats, in_=x_t)
                else:
                    xr = x_t.rearrange("p (s f) -> p s f", f=fmax)
                    stats = stat.tile([P, nsub, 6], f32, tag="bns", bufs=6,
                                      name="st6")
                    for s in range(nsub):
                        nc.vector.bn_stats(out=stats[:, s], in_=xr[:, s])
                mv = stat.tile([P, 2], f32, tag="bna", bufs=8, name="mv")
                nc.vector.bn_aggr(out=mv, in_=stats)
                xs.append(x_t)
                mvs.append(mv)
            state["xs"], state["mvs"] = xs, mvs

        def ln_act(state):
            mvs = state["mvs"]
            lnv = stat.tile([P, 4], f32, tag="lnv", name="lnv")
            for tl in range(4):
                nc.scalar.activation(
                    out=lnv[:, tl : tl + 1], in_=mvs[tl][:, 1:2], func=AF.Ln,
                    bias=eps_t, scale=1.0,
                )
            rstd = stat.tile([P, 4], f32, tag="rstd", name="rstd")
            nc.scalar.activation(out=rstd, in_=lnv, func=AF.Exp, scale=-0.5)
            nmr = stat.tile([P, 4], f32, tag="nmr", name="nmr")
            for tl in range(4):
                nc.vector.scalar_tensor_tensor(
                    out=nmr[:, tl : tl + 1], in0=mvs[tl][:, 0:1], scalar=-1.0,
                    in1=rstd[:, tl : tl + 1], op0=OP.mult, op1=OP.mult,
                )
            state["rstd"], state["nmr"] = rstd, nmr

        def ln_cen(width, state):
            cents = []
            for tl in range(4):
                cen = cenp.tile([P, width], bf16, tag="cen", name="cen")
                nc.vector.tensor_scalar(
                    out=cen, in0=state["xs"][tl],
                    scalar1=state["rstd"][:, tl : tl + 1],
                    scalar2=state["nmr"][:, tl : tl + 1],
                    op0=OP.mult, op1=OP.add,
                )
                cents.append(cen)
            state["cents"] = cents

        def ln_tp(tg, fc, g_strip, b_strip, dst, state):
            # Transpose on the DMA xbar, not the PE: 128 transposes would
            # cost ~35us of tensor-engine time the kernel can't spare.
            tp = cenp.tile([P, 4, P], bf16, tag="tpsb", bufs=4, name="tp")
            for tl in range(4):
                nc.sync.dma_start_transpose(
                    out=tp[:, tl], in_=state["cents"][tl][:, fc * P : (fc + 1) * P]
                )
            nc.vector.tensor_scalar(
                out=dst[:, fc, tg * 512 : (tg + 1) * 512], in0=tp,
                scalar1=g_strip[:, fc : fc + 1], scalar2=b_strip[:, fc : fc + 1],
                op0=OP.mult, op1=OP.add,
            )

        xsrc = io["x"].rearrange("(t p) d -> p t d", p=P)
        csrc = io["cond"].rearrange("(t p) d -> p t d", p=P)

        def ln_group_thunks(src, width, tg, g_strip, b_strip, dst):
            state = {}
            fc_n = width // P
            out = [lambda: ln_dma(src, width, tg, state),
                   lambda: ln_act(state),
                   lambda: ln_cen(width, state)]
            for fc in range(fc_n):
                out.append(
                    lambda fc=fc: ln_tp(tg, fc, g_strip, b_strip, dst, state)
                )
            return out

        # ---- projection thunks -----------------------------------------
        def k_proj(m, ng):
            acc = ps.tile([P, 512], f32, tag="bk", bufs=4, name="acck")
            for k in range(FC_C):
                nc.tensor.matmul(
                    acc, lhsT=wk_b[:, k, m * P : (m + 1) * P],
                    rhs=cnT[:, k, ng * 512 : (ng + 1) * 512],
                    start=(k == 0), stop=(k == FC_C - 1),
                )
            nc.vector.tensor_copy(
                out=KT[:, m, ng * 512 : (ng + 1) * 512], in_=acc
            )

        def q_proj(m, nt):
            acc = ps.tile([P, 512], f32, tag="bk", bufs=4, name="accq")
            for k in range(FC_X):
                nc.tensor.matmul(
                    acc, lhsT=wq_b[:, k, m * P : (m + 1) * P],
                    rhs=xnT[:, k, nt * 512 : (nt + 1) * 512],
                    start=(k == 0), stop=(k == FC_X - 1),
                )
            nc.vector.tensor_scalar(
                out=QT[:, m, nt * 512 : (nt + 1) * 512], in0=acc,
                scalar1=SCALE, scalar2=None, op0=OP.mult,
            )

        def v_proj(mc):
            acc = ps.tile([P, 512], f32, tag="bk", bufs=4, name="accv")
            for k in range(FC_C):
                nc.tensor.matmul(
                    acc, lhsT=cnT[:, k, mc * P : (mc + 1) * P],
                    rhs=wv_b[:, k, :], start=(k == 0), stop=(k == FC_C - 1),
                )
            if FP8_PV:
                dst = V8[:, mc // 2, mc % 2, :, 0:DH]
            else:
                dst = V8[:, mc, :, 0:DH]
            nc.vector.tensor_copy(
                out=dst, in_=acc.rearrange("p (h d) -> p h d", h=H)
            )

        # ---- residual x preload ----------------------------------------
        xres = xresp.tile([P, 8, DQ], bf16, name="xres")

        def xres_load(t):
            nc.gpsimd.dma_start(out=xres[:, t], in_=xsrc[:, t])
            nc.gpsimd.tensor_add(out=xres[:, t], in0=xres[:, t], in1=bo_bc)

        # ---- Wo + residual + final LN per 128-token block --------------
        outr = io["out"].rearrange("(t p) d -> p t d", p=P)

        # Wo is split: wo_mm (PE chain + residual + stats, no ACT) paces as
        # PE filler; wo_fin (the two tiny ACT ops + apply + store) is
        # flushed at attention-block boundaries so it never head-of-line
        # blocks the exp stream on the FIFO ACT queue.
        wo_state = {}
        wo_pending = []

        def wo_mm(t):
            y_ps = ps.tile([P, 512], f32, tag="bk", bufs=4, name="y_ps")
            for h in range(H):
                nc.tensor.matmul(
                    y_ps, lhsT=OT[:, h, t * P : (t + 1) * P],
                    rhs=wo_b[:, h, :], start=(h == 0), stop=(h == H - 1),
                )
            y1 = work.tile([P, DQ], f32, tag="y1", bufs=4, name="y1")
            nc.vector.tensor_add(out=y1, in0=y_ps, in1=xres[:, t])
            stats = stat.tile([P, 6], f32, tag="bns", bufs=6, name="stf")
            nc.vector.bn_stats(out=stats, in_=y1)
            mv = stat.tile([P, 2], f32, tag="bna", bufs=8, name="mvf")
            nc.vector.bn_aggr(out=mv, in_=stats)
            wo_state[t] = (y1, mv)
            wo_pending.append(t)

        def wo_fin(t):
            y1, mv = wo_state.pop(t)
            lv = stat.tile([P, 1], f32, tag="lvf", bufs=4, name="lvf")
            nc.scalar.activation(
                out=lv, in_=mv[:, 1:2], func=AF.Ln, bias=eps_t, scale=1.0
            )
            rstdf = stat.tile([P, 1], f32, tag="rstf", bufs=4, name="rstdf")
            nc.scalar.activation(out=rstdf, in_=lv, func=AF.Exp, scale=-0.5)
            nc.vector.tensor_scalar(
                out=y1, in0=y1, scalar1=mv[:, 0:1], scalar2=rstdf,
                op0=OP.subtract, op1=OP.mult,
            )
            nc.vector.tensor_mul(out=y1, in0=y1, in1=gf_bc)
            nc.gpsimd.tensor_add(out=y1, in0=y1, in1=bf_bc)
            nc.sync.dma_start(out=outr[:, t], in_=y1)

        def wo_flush():
            while wo_pending:
                wo_fin(wo_pending.pop(0))

        # ---- attention -------------------------------------------------
        def normalize(ot, h, nt):
            # 1/d = exp(-ln(d)) keeps the reciprocal on ACT's ln/exp table
            # (DVE's iterative-divide reciprocal costs ~8 cycles/element).
            rb = work.tile([P, 512], f32, tag="rb", bufs=2, name="rb")
            nc.scalar.activation(
                out=rb[DH : DH + 1, :], in_=ot[DH : DH + 1, :], func=AF.Ln
            )
            nc.scalar.activation(
                out=rb[DH : DH + 1, :], in_=rb[DH : DH + 1, :], func=AF.Exp,
                scale=-1.0,
            )
            r0 = work.tile([1, 512], f32, tag="r0", bufs=2, name="r0")
            nc.sync.dma_start(out=r0, in_=rb[DH : DH + 1, :])
            nc.gpsimd.partition_broadcast(rb[0:DH, :], r0[0:1, :])
            nc.vector.tensor_mul(
                out=OT[:, h, nt * 512 : (nt + 1) * 512],
                in0=ot[0:DH, :], in1=rb[0:DH, :],
            )

        # Background task graph.  Tile dependencies follow PROGRAM order,
        # so a producer must be *emitted* before its consumer.  `ensure`
        # emits a task (after its declared prerequisites); `pace` trickles
        # the remaining (deferrable) tasks across the attention blocks so
        # the PE always has filler work under the ACT-bound exp stream.
        tasks = {}
        order = []
        done = set()

        def task(key, deps, th):
            tasks[key] = (deps, th)
            order.append(key)

        def ensure(key):
            if key in done or key not in tasks:
                return
            done.add(key)
            deps, th = tasks[key]
            for dkey in deps:
                ensure(dkey)
            th()

        pace_pos = [0]

        def pace(n):
            count = 0
            while count < n and pace_pos[0] < len(order):
                key = order[pace_pos[0]]
                pace_pos[0] += 1
                if key not in done:
                    ensure(key)
                    count += 1

        def attention(c, nt):
            hA, hB = 2 * c, 2 * c + 1
            q_a = QT[0:64, c, nt * 512 : (nt + 1) * 512]
            q_b = QT[64:128, c, nt * 512 : (nt + 1) * 512]
            otA = ps.tile([P, 512], f32, tag="bk", bufs=4, name="otA")
            otB = ps.tile([P, 512], f32, tag="bk", bufs=4, name="otB")

            def emit_pv(g, pA, pB):
                if FP8_PV:
                    nc.tensor.matmul(
                        otA[0 : DH + 1, :], lhsT=V8[:, g, :, hA, 0 : DH + 1],
                        rhs=pA, start=(g == 0), stop=(g == TK2 - 1),
                        perf_mode=PM.DoubleRow,
                    )
                    nc.tensor.matmul(
                        otB[0 : DH + 1, :], lhsT=V8[:, g, :, hB, 0 : DH + 1],
                        rhs=pB, start=(g == 0), stop=(g == TK2 - 1),
                        perf_mode=PM.DoubleRow,
                    )
                else:
                    for j in range(2):
                        mc = 2 * g + j
                        nc.tensor.matmul(
                            otA[0 : DH + 1, :], lhsT=V8[:, mc, hA, :],
                            rhs=pA[:, j], start=(mc == 0), stop=(mc == TK - 1),
                        )
                        nc.tensor.matmul(
                            otB[0 : DH + 1, :], lhsT=V8[:, mc, hB, :],
                            rhs=pB[:, j], start=(mc == 0), stop=(mc == TK - 1),
                        )

            pend = None
            for g in range(TK2):
                ensure(("k", c, g // 2))
                ensure(("v", 2 * g))
                ensure(("v", 2 * g + 1))
                # One st tile per head holds both chunks of the group, so
                # head A's tile recycles while head B's exp drains: the
                # exp->scores->exp recycle path hides under the other
                # head's exp instead of serializing the stream.
                stA = ps.tile([P, 2, 512], f32, tag="st", bufs=2, name="stA")
                stB = ps.tile([P, 2, 512], f32, tag="st", bufs=2, name="stB")
                # Alternate the row-tiles (A at rows 0:63, B at 64:127) so
                # every adjacent matmul pair targets disjoint tiles and can
                # co-issue in the PE array.
                for j in range(2):
                    mc = 2 * g + j
                    nc.tensor.matmul(
                        stA[:, j], lhsT=KT[0:64, c, mc * P : (mc + 1) * P],
                        rhs=q_a, start=True, stop=True, tile_position=(0, 0),
                    )
                    nc.tensor.matmul(
                        stB[:, j], lhsT=KT[64:128, c, mc * P : (mc + 1) * P],
                        rhs=q_b, start=True, stop=True, tile_position=(64, 0),
                    )
                pA = pp.tile([P, 2, 512], pdt, tag="p", name="pA")
                nc.scalar.activation(out=pA, in_=stA, func=AF.Exp, bias=esh_t)
                pB = pp.tile([P, 2, 512], pdt, tag="p", name="pB")
                nc.scalar.activation(out=pB, in_=stB, func=AF.Exp, bias=esh_t)
                if pend is not None:
                    emit_pv(*pend)
                pend = (g, pA, pB)
                pace(1)
            emit_pv(*pend)
            normalize(otA, hA, nt)
            normalize(otB, hB, nt)

        # ---- emission schedule -----------------------------------------
        # PE warmup: dummy matmuls fill the DMA-bound lead-in so the HAM
        # clock gate reaches 8/8 before the first real matmuls arrive.
        dummy = singles.tile([P, 512], bf16, name="dummy")
        nc.vector.memset(dummy, 0.5)
        for _ in range(24):
            wu = ps.tile([P, 512], f32, tag="bk", bufs=4, name="wu")
            nc.tensor.matmul(wu, lhsT=ident, rhs=dummy, start=True, stop=True)

        # upfront: everything attention(c=0, nt=0) groups 0..3 need; the
        # activation DMAs are emitted before the weight stages so the
        # input tiles win the DMA queues.
        for tg in (0, 1):
            for th in ln_group_thunks(csrc, DC, tg, gc, bc_, cnT):
                th()
        for th in ln_group_thunks(xsrc, DQ, 0, gx, bx, xnT):
            th()
        wq_b = load_weight(io["Wq"], DQ, "wq_b")
        wk_b = load_weight(io["Wk"], DC, "wk_b")
        wv_b = load_weight(io["Wv"], DC, "wv_b")
        k_proj(0, 0)
        k_proj(0, 1)
        q_proj(0, 0)
        for mc in range(4):
            v_proj(mc)
        wo_b = load_wo()

        # deferrable/background tasks with explicit producer deps; `order`
        # doubles as the pacing sequence (roughly consumption order).
        CL = 8  # last thunk index of a cond LN chain (9 thunks)
        for t in range(8):
            task(("xr", t), [], lambda t=t: xres_load(t))
        for tg in (2, 3):
            ths = ln_group_thunks(csrc, DC, tg, gc, bc_, cnT)
            prev = []
            for i, th in enumerate(ths):
                task(("c", tg, i), prev, th)
                prev = [("c", tg, i)]
            task(("k", 0, tg), prev, lambda ng=tg: k_proj(0, ng))
            for mc in range(4 * tg - 4, 4 * tg):
                task(("v", mc), [("c", mc // 4, CL)], lambda mc=mc: v_proj(mc))
        for mc in (12, 13, 14, 15):
            task(("v", mc), [("c", 3, CL)], lambda mc=mc: v_proj(mc))
        ths = ln_group_thunks(xsrc, DQ, 1, gx, bx, xnT)
        prev = []
        for i, th in enumerate(ths):
            task(("x1", i), prev, th)
            prev = [("x1", i)]
        x1_last = prev
        for m in (1, 2, 3):
            task(("q", m, 0), [], lambda m=m: q_proj(m, 0))
            for ng in range(4):
                task(("k", m, ng), [("c", ng, CL)],
                     lambda m=m, ng=ng: k_proj(m, ng))
        for m in range(4):
            task(("q", m, 1), x1_last, lambda m=m: q_proj(m, 1))

        # Block order interleaves the two query tiles so Wo(nt=0) work is
        # available as late PE filler.
        blocks = [(0, 0), (0, 1), (0, 2), (1, 0), (0, 3), (1, 1), (1, 2),
                  (1, 3)]
        for nt, c in blocks:
            ensure(("q", c, nt))
            attention(c, nt)
            wo_flush()
            if (nt, c) == (0, 3):
                for t in range(4):
                    task(("wo0", t), [("xr", t)], lambda t=t: wo_mm(t))
        pace(len(order))
        wo_flush()
        for t in range(4, 8):
            ensure(("xr", t))
            wo_mm(t)
        wo_flush()


def _pin_act_table_set():
    """Make Exp and Ln resolve to the one table set that holds both
    (natural_log_exp_and_others) so the kernel never thrashes ACT table
    loads between exp-only and ln-only sets.  Only the selection metadata
    is filtered; set ids stay aligned with act_info.json."""
    import concourse.bacc as bacc
    import concourse.mybir as mybir

    if getattr(bacc, "_act_tables_pinned", False):
        return
    AF = mybir.ActivationFunctionType
    orig = bacc.get_activation_tables

    def patched(arch):
        tables = dict(orig(arch))
        for name, fns in tables.items():
            if name != "natural_log_exp_and_others":
                fns.discard(AF.Exp)
                fns.discard(AF.Ln)
        return tables

    bacc.get_activation_tables = patched
    bacc._act_tables_pinned = True


@functools.cache
def _build_program():
    import concourse.bacc as bacc
    import concourse.mybir as mybir
    import concourse.tile as tile

    _pin_act_table_set()

    f32 = mybir.dt.float32
    nc = bacc.Bacc()
    io = {}
    io["x"] = nc.declare_dram_parameter("x", [NQ, DQ], f32, False)[:, :]
    io["cond"] = nc.declare_dram_parameter("cond", [M, DC], f32, False)[:, :]
    for name in ("lnx_g", "lnx_b"):
        io[name] = nc.declare_dram_parameter(name, [DQ], f32, False)[:]
    for name in ("lnc_g", "lnc_b"):
        io[name] = nc.declare_dram_parameter(name, [DC], f32, False)[:]
    io["Wq"] = nc.declare_dram_parameter("Wq", [DQ, INNER], f32, False)[:, :]
    io["Wk"] = nc.declare_dram_parameter("Wk", [DC, INNER], f32, False)[:, :]
    io["Wv"] = nc.declare_dram_parameter("Wv", [DC, INNER], f32, False)[:, :]
    io["Wo"] = nc.declare_dram_parameter("Wo", [INNER, DQ], f32, False)[:, :]
    for name in ("bo", "lnf_g", "lnf_b"):
        io[name] = nc.declare_dram_parameter(name, [DQ], f32, False)[:]
    io["out"] = nc.declare_dram_parameter("out", [NQ, DQ], f32, True)[:, :]

    with tile.TileContext(nc) as tc:
        _emit(tc, io)
    nc.compile()
    return nc


def _core_input_map(inputs, core):
    b, half = core // 2, core % 2
    m = {
        "x": np.ascontiguousarray(inputs["x"][b, half * NQ : (half + 1) * NQ]),
        "cond": np.ascontiguousarray(inputs["cond"][b]),
    }
    for name in (
        "lnx_g",
        "lnx_b",
        "lnc_g",
        "lnc_b",
        "Wq",
        "Wk",
        "Wv",
        "Wo",
        "bo",
        "lnf_g",
        "lnf_b",
    ):
        m[name] = np.asarray(inputs[name], dtype=np.float32)
    return m


TRACE = False
LAST_RESULTS = None


def kernel(**inputs):
    from concourse.bass_utils import run_bass_kernel_spmd

    global LAST_RESULTS
    nc = _build_program()
    in_maps = [_core_input_map(inputs, core) for core in range(N_CORES)]
    res = run_bass_kernel_spmd(
        nc,
        in_maps,
        list(range(N_CORES)),
        trace=TRACE,
        trace_cores=[0] if TRACE else None,
    )
    LAST_RESULTS = res
    out = np.empty((B, N, DQ), np.float32)
    for core in range(N_CORES):
        b, half = core // 2, core % 2
        out[b, half * NQ : (half + 1) * NQ] = res.results[core]["out"]
    return out


# revision 28
# speedup vs baseline: 1.5217x; 1.5217x over previous
"""Cross-attention block on 8 Trainium2 NeuronCores (v2).

Computes, per batch b:
    xn = LN(x); cn = LN(cond)
    q = xn @ Wq; k = cn @ Wk; v = cn @ Wv   (8 heads x 64)
    out = softmax(q k^T / sqrt(64)) v
    y  = LN(out @ Wo + bo + x)

Sharding: 8 cores = 4 batches x 2 query-row halves (data parallel over
(batch, query-block)), no collectives.

v2 structure (vs the v1 baseline):
  * Scores are computed as row-tiled matmul PAIRS: head A's K-chunk loads
    into PE rows 0:63 (tile_position (0,0)) and head B's into rows 64:127
    ((64,0)); the two 512-col streams co-issue, halving score time.
  * PV uses fp8(e4m3) DoubleRow: the stationary holds 2 keys per cell
    ([128, 2, 65] V-chunks), the moving operand is exp(scores) written by
    ACT directly in fp8 as [128, 2, 512] double-chunks.  256 keys per
    matmul -> half the PV streams.  exp carries a free bias of -1.5 so
    values stay inside e4m3 range; the shift cancels in softmax.
  * The softmax denominator stays fused as a 65th ones-column of V.
  * All LN rstd values use exp(-0.5*ln(var+eps)) so the single ACT table
    set (natural_log_exp) serves the whole kernel - no table switches.
  * LN centering/scaling runs on DVE (tensor_scalar), not ACT: ACT is the
    critical engine (exp stream ~128us) and does nothing else during
    attention.
  * Matmuls are grouped by PE tiling mode (64-row score groups vs 128-row
    PV/projection groups) to amortize the mode-switch drain.
  * Projections / LN transposes / Wo / residual loads are emitted as
    background thunks interleaved between attention groups so the Tile
    scheduler can fill PE slack under the ACT-bound exp stream and keep
    the PE HAM-warm.
"""

import functools

import numpy as np

B, N, M = 4, 2048, 2048
DQ, DC = 512, 768
H, DH = 8, 64
INNER = H * DH  # 512
P = 128
NQ = N // 2  # query rows per core
EPS = 1e-5
N_CORES = 8

FC_X = DQ // P  # 4 feature chunks of x
FC_C = DC // P  # 6 feature chunks of cond
IC = INNER // P  # 4 inner chunks (= head pairs)
TK = M // P  # 16 key-token chunks
TK2 = TK // 2  # 8 double (256-key) chunks
NT = NQ // 512  # 2 query column tiles
VP = 80  # padded fp8 V row stride (step%16==0)
SCALE = float(DH) ** -0.5
ESHIFT = -1.0  # exp range shift for fp8; cancels in softmax

FP8_PV = True


def _emit(tc, io):
    import contextlib
    import math

    import concourse.bass as bass
    import concourse.mybir as mybir

    nc = tc.nc
    f32 = mybir.dt.float32
    bf16 = mybir.dt.bfloat16
    f8 = mybir.dt.float8e4
    AF = mybir.ActivationFunctionType
    OP = mybir.AluOpType
    PM = mybir.MatmulPerfMode

    pdt = f8 if FP8_PV else bf16

    ctx = contextlib.ExitStack()
    with ctx:
        singles = ctx.enter_context(tc.tile_pool(name="singles", bufs=1))
        wst = ctx.enter_context(tc.tile_pool(name="wst", bufs=1))
        work = ctx.enter_context(tc.tile_pool(name="work", bufs=3))
        stat = ctx.enter_context(tc.tile_pool(name="stat", bufs=4))
        cenp = ctx.enter_context(tc.tile_pool(name="cenp", bufs=5))
        pp = ctx.enter_context(tc.tile_pool(name="pp", bufs=4))
        xresp = ctx.enter_context(tc.tile_pool(name="xresp", bufs=1))
        ps = ctx.enter_context(tc.tile_pool(name="ps", bufs=1, space="PSUM"))

        # ---- constants -------------------------------------------------
        from concourse.masks import make_identity

        ident = singles.tile([P, P], bf16, name="ident")
        make_identity(nc, ident)
        eps_t = singles.tile([P, 1], f32, name="eps_t")
        nc.vector.memset(eps_t, EPS)
        esh_t = singles.tile([P, 1], f32, name="esh_t")
        nc.vector.memset(esh_t, ESHIFT)

        def bcast_load(vec_ap, width, name):
            t = singles.tile([P, width], f32, name=name)
            bc = bass.AP(
                tensor=vec_ap.tensor,
                offset=vec_ap.offset,
                ap=[[0, P]] + [list(a) for a in vec_ap.ap],
            )
            nc.gpsimd.dma_start(out=t, in_=bc)
            return t

        def strip_load(vec_ap, chunks, name):
            t = singles.tile([P, chunks], f32, name=name)
            nc.sync.dma_start(out=t, in_=vec_ap.rearrange("(c p) -> p c", p=P))
            return t

        gx = strip_load(io["lnx_g"], FC_X, "gx")
        bx = strip_load(io["lnx_b"], FC_X, "bx")
        gc = strip_load(io["lnc_g"], FC_C, "gc")
        bc_ = strip_load(io["lnc_b"], FC_C, "bc")
        gf_bc = bcast_load(io["lnf_g"], DQ, "gf_bc")
        bf_bc = bcast_load(io["lnf_b"], DQ, "bf_bc")
        bo_bc = bcast_load(io["bo"], DQ, "bo_bc")

        # ---- weights: fp32 HBM -> bf16 SBUF, contraction on partitions --
        def load_weight(w_ap, din, name):
            kc = din // P
            stage = wst.tile([P, kc, INNER], f32, tag="wstage", bufs=1,
                             name=f"{name}_st")
            nc.sync.dma_start(
                out=stage, in_=w_ap.rearrange("(ko p) i -> p ko i", p=P)
            )
            wb = singles.tile([P, kc, INNER], bf16, name=name)
            nc.scalar.copy(out=wb, in_=stage)
            return wb

        def load_wo():
            wo_st = wst.tile([DH, H, DQ], f32, tag="wostage", name="wo_st")
            nc.sync.dma_start(
                out=wo_st, in_=io["Wo"].rearrange("(h p) d -> p h d", p=DH)
            )
            wo_b = singles.tile([DH, H, DQ], bf16, name="wo_b")
            nc.vector.tensor_copy(out=wo_b, in_=wo_st)
            return wo_b

        # ---- persistent activations ------------------------------------
        xnT = singles.tile([P, FC_X, NQ], bf16, name="xnT")
        cnT = singles.tile([P, FC_C, M], bf16, name="cnT")
        QT = singles.tile([P, IC, NQ], bf16, name="QT")
        KT = singles.tile([P, IC, M], bf16, name="KT")
        if FP8_PV:
            # V8[:, g, j, h, 0:64] = v for key chunk 2g+j; [..., 64] = 1
            # (fused softmax denominator); cols 65:80 pad for step%16==0.
            V8 = singles.tile([P, TK2, 2, H, VP], f8, name="V8")
            nc.vector.memset(V8, 1.0)
        else:
            V8 = singles.tile([P, TK, H, DH + 1], bf16, name="V8")
            nc.vector.memset(V8, 1.0)
        OT = singles.tile([DH, H, NQ], bf16, name="OT")

        # ---- LN + PE-transpose into feature-major layout ---------------
        def ln_dma(src, width, tg, state):
            """DMA 4 token-chunks + stats (no ACT work: the ACT queue is
            strict FIFO and must never wait on a fresh DMA mid-attention)."""
            fmax = math.gcd(512, width)
            nsub = width // fmax
            xs, mvs = [], []
            for tl in range(4):
                x_t = work.tile([P, width], f32, tag="xin", bufs=4, name="x_t")
                nc.sync.dma_start(out=x_t, in_=src[:, tg * 4 + tl])
                if nsub == 1:
                    stats = stat.tile([P, 6], f32, tag="bns", bufs=6, name="st6")
                    nc.vector.bn_stats(out=stats, in_=x_t)
                else:
                    xr = x_t.rearrange("p (s f) -> p s f", f=fmax)
                    stats = stat.tile([P, nsub, 6], f32, tag="bns", bufs=6,
                                      name="st6")
                    for s in range(nsub):
                        nc.vector.bn_stats(out=stats[:, s], in_=xr[:, s])
                mv = stat.tile([P, 2], f32, tag="bna", bufs=8, name="mv")
                nc.vector.bn_aggr(out=mv, in_=stats)
                xs.append(x_t)
                mvs.append(mv)
            state["xs"], state["mvs"] = xs, mvs

        def ln_act(state):
            mvs = state["mvs"]
            lnv = stat.tile([P, 4], f32, tag="lnv", name="lnv")
            for tl in range(4):
                nc.scalar.activation(
                    out=lnv[:, tl : tl + 1], in_=mvs[tl][:, 1:2], func=AF.Ln,
                    bias=eps_t, scale=1.0,
                )
            rstd = stat.tile([P, 4], f32, tag="rstd", name="rstd")
            nc.scalar.activation(out=rstd, in_=lnv, func=AF.Exp, scale=-0.5)
            nmr = stat.tile([P, 4], f32, tag="nmr", name="nmr")
            for tl in range(4):
                nc.vector.scalar_tensor_tensor(
                    out=nmr[:, tl : tl + 1], in0=mvs[tl][:, 0:1], scalar=-1.0,
                    in1=rstd[:, tl : tl + 1], op0=OP.mult, op1=OP.mult,
                )
            state["rstd"], state["nmr"] = rstd, nmr

        def ln_cen(width, state):
            cents = []
            for tl in range(4):
                cen = cenp.tile([P, width], bf16, tag="cen", name="cen")
                nc.vector.tensor_scalar(
                    out=cen, in0=state["xs"][tl],
                    scalar1=state["rstd"][:, tl : tl + 1],
                    scalar2=state["nmr"][:, tl : tl + 1],
                    op0=OP.mult, op1=OP.add,
                )
                cents.append(cen)
            state["cents"] = cents

        def ln_tp(tg, fc, g_strip, b_strip, dst, state):
            tp = ps.tile([P, 4, P], bf16, tag="bk", bufs=4, name="tp")
            for tl in range(4):
                nc.tensor.transpose(
                    tp[:, tl], state["cents"][tl][:, fc * P : (fc + 1) * P],
                    ident,
                )
            nc.vector.tensor_scalar(
                out=dst[:, fc, tg * 512 : (tg + 1) * 512], in0=tp,
                scalar1=g_strip[:, fc : fc + 1], scalar2=b_strip[:, fc : fc + 1],
                op0=OP.mult, op1=OP.add,
            )

        xsrc = io["x"].rearrange("(t p) d -> p t d", p=P)
        csrc = io["cond"].rearrange("(t p) d -> p t d", p=P)

        def ln_group_thunks(src, width, tg, g_strip, b_strip, dst):
            state = {}
            fc_n = width // P
            out = [lambda: ln_dma(src, width, tg, state),
                   lambda: ln_act(state),
                   lambda: ln_cen(width, state)]
            for fc in range(fc_n):
                out.append(
                    lambda fc=fc: ln_tp(tg, fc, g_strip, b_strip, dst, state)
                )
            return out

        # ---- projection thunks -----------------------------------------
        def k_proj(m, ng):
            acc = ps.tile([P, 512], f32, tag="bk", bufs=4, name="acck")
            for k in range(FC_C):
                nc.tensor.matmul(
                    acc, lhsT=wk_b[:, k, m * P : (m + 1) * P],
                    rhs=cnT[:, k, ng * 512 : (ng + 1) * 512],
                    start=(k == 0), stop=(k == FC_C - 1),
                )
            nc.vector.tensor_copy(
                out=KT[:, m, ng * 512 : (ng + 1) * 512], in_=acc
            )

        def q_proj(m, nt):
            acc = ps.tile([P, 512], f32, tag="bk", bufs=4, name="accq")
            for k in range(FC_X):
                nc.tensor.matmul(
                    acc, lhsT=wq_b[:, k, m * P : (m + 1) * P],
                    rhs=xnT[:, k, nt * 512 : (nt + 1) * 512],
                    start=(k == 0), stop=(k == FC_X - 1),
                )
            nc.vector.tensor_scalar(
                out=QT[:, m, nt * 512 : (nt + 1) * 512], in0=acc,
                scalar1=SCALE, scalar2=None, op0=OP.mult,
            )

        def v_proj(mc):
            acc = ps.tile([P, 512], f32, tag="bk", bufs=4, name="accv")
            for k in range(FC_C):
                nc.tensor.matmul(
                    acc, lhsT=cnT[:, k, mc * P : (mc + 1) * P],
                    rhs=wv_b[:, k, :], start=(k == 0), stop=(k == FC_C - 1),
                )
            if FP8_PV:
                dst = V8[:, mc // 2, mc % 2, :, 0:DH]
            else:
                dst = V8[:, mc, :, 0:DH]
            nc.vector.tensor_copy(
                out=dst, in_=acc.rearrange("p (h d) -> p h d", h=H)
            )

        # ---- residual x preload ----------------------------------------
        xres = xresp.tile([P, 8, DQ], bf16, name="xres")

        def xres_load(t):
            nc.gpsimd.dma_start(out=xres[:, t], in_=xsrc[:, t])
            nc.gpsimd.tensor_add(out=xres[:, t], in0=xres[:, t], in1=bo_bc)

        # ---- Wo + residual + final LN per 128-token block --------------
        outr = io["out"].rearrange("(t p) d -> p t d", p=P)

        # Wo is split: wo_mm (PE chain + residual + stats, no ACT) paces as
        # PE filler; wo_fin (the two tiny ACT ops + apply + store) is
        # flushed at attention-block boundaries so it never head-of-line
        # blocks the exp stream on the FIFO ACT queue.
        wo_state = {}
        wo_pending = []

        def wo_mm(t):
            y_ps = ps.tile([P, 512], f32, tag="bk", bufs=4, name="y_ps")
            for h in range(H):
                nc.tensor.matmul(
                    y_ps, lhsT=OT[:, h, t * P : (t + 1) * P],
                    rhs=wo_b[:, h, :], start=(h == 0), stop=(h == H - 1),
                )
            y1 = work.tile([P, DQ], f32, tag="y1", bufs=4, name="y1")
            nc.vector.tensor_add(out=y1, in0=y_ps, in1=xres[:, t])
            stats = stat.tile([P, 6], f32, tag="bns", bufs=6, name="stf")
            nc.vector.bn_stats(out=stats, in_=y1)
            mv = stat.tile([P, 2], f32, tag="bna", bufs=8, name="mvf")
            nc.vector.bn_aggr(out=mv, in_=stats)
            wo_state[t] = (y1, mv)
            wo_pending.append(t)

        def wo_fin(t):
            y1, mv = wo_state.pop(t)
            lv = stat.tile([P, 1], f32, tag="lvf", bufs=4, name="lvf")
            nc.scalar.activation(
                out=lv, in_=mv[:, 1:2], func=AF.Ln, bias=eps_t, scale=1.0
            )
            rstdf = stat.tile([P, 1], f32, tag="rstf", bufs=4, name="rstdf")
            nc.scalar.activation(out=rstdf, in_=lv, func=AF.Exp, scale=-0.5)
            nc.vector.tensor_scalar(
                out=y1, in0=y1, scalar1=mv[:, 0:1], scalar2=rstdf,
                op0=OP.subtract, op1=OP.mult,
            )
            nc.vector.tensor_mul(out=y1, in0=y1, in1=gf_bc)
            nc.gpsimd.tensor_add(out=y1, in0=y1, in1=bf_bc)
            nc.sync.dma_start(out=outr[:, t], in_=y1)

        def wo_flush():
            while wo_pending:
                wo_fin(wo_pending.pop(0))

        # ---- attention -------------------------------------------------
        def normalize(ot, h, nt):
            # 1/d = exp(-ln(d)) keeps the reciprocal on ACT's ln/exp table
            # (DVE's iterative-divide reciprocal costs ~8 cycles/element).
            rb = work.tile([P, 512], f32, tag="rb", bufs=2, name="rb")
            nc.scalar.activation(
                out=rb[DH : DH + 1, :], in_=ot[DH : DH + 1, :], func=AF.Ln
            )
            nc.scalar.activation(
                out=rb[DH : DH + 1, :], in_=rb[DH : DH + 1, :], func=AF.Exp,
                scale=-1.0,
            )
            r0 = work.tile([1, 512], f32, tag="r0", bufs=2, name="r0")
            nc.sync.dma_start(out=r0, in_=rb[DH : DH + 1, :])
            nc.gpsimd.partition_broadcast(rb[0:DH, :], r0[0:1, :])
            nc.vector.tensor_mul(
                out=OT[:, h, nt * 512 : (nt + 1) * 512],
                in0=ot[0:DH, :], in1=rb[0:DH, :],
            )

        # Background task graph.  Tile dependencies follow PROGRAM order,
        # so a producer must be *emitted* before its consumer.  `ensure`
        # emits a task (after its declared prerequisites); `pace` trickles
        # the remaining (deferrable) tasks across the attention blocks so
        # the PE always has filler work under the ACT-bound exp stream.
        tasks = {}
        order = []
        done = set()

        def task(key, deps, th):
            tasks[key] = (deps, th)
            order.append(key)

        def ensure(key):
            if key in done or key not in tasks:
                return
            done.add(key)
            deps, th = tasks[key]
            for dkey in deps:
                ensure(dkey)
            th()

        pace_pos = [0]

        def pace(n):
            count = 0
            while count < n and pace_pos[0] < len(order):
                key = order[pace_pos[0]]
                pace_pos[0] += 1
                if key not in done:
                    ensure(key)
                    count += 1

        def attention(c, nt):
            hA, hB = 2 * c, 2 * c + 1
            q_a = QT[0:64, c, nt * 512 : (nt + 1) * 512]
            q_b = QT[64:128, c, nt * 512 : (nt + 1) * 512]
            otA = ps.tile([P, 512], f32, tag="bk", bufs=4, name="otA")
            otB = ps.tile([P, 512], f32, tag="bk", bufs=4, name="otB")

            def emit_pv(g, pA, pB):
                if FP8_PV:
                    nc.tensor.matmul(
                        otA[0 : DH + 1, :], lhsT=V8[:, g, :, hA, 0 : DH + 1],
                        rhs=pA, start=(g == 0), stop=(g == TK2 - 1),
                        perf_mode=PM.DoubleRow,
                    )
                    nc.tensor.matmul(
                        otB[0 : DH + 1, :], lhsT=V8[:, g, :, hB, 0 : DH + 1],
                        rhs=pB, start=(g == 0), stop=(g == TK2 - 1),
                        perf_mode=PM.DoubleRow,
                    )
                else:
                    for j in range(2):
                        mc = 2 * g + j
                        nc.tensor.matmul(
                            otA[0 : DH + 1, :], lhsT=V8[:, mc, hA, :],
                            rhs=pA[:, j], start=(mc == 0), stop=(mc == TK - 1),
                        )
                        nc.tensor.matmul(
                            otB[0 : DH + 1, :], lhsT=V8[:, mc, hB, :],
                            rhs=pB[:, j], start=(mc == 0), stop=(mc == TK - 1),
                        )

            pend = None
            for g in range(TK2):
                ensure(("k", c, g // 2))
                ensure(("v", 2 * g))
                ensure(("v", 2 * g + 1))
                # One st tile per head holds both chunks of the group, so
                # head A's tile recycles while head B's exp drains: the
                # exp->scores->exp recycle path hides under the other
                # head's exp instead of serializing the stream.
                stA = ps.tile([P, 2, 512], f32, tag="st", bufs=2, name="stA")
                stB = ps.tile([P, 2, 512], f32, tag="st", bufs=2, name="stB")
                # Alternate the row-tiles (A at rows 0:63, B at 64:127) so
                # every adjacent matmul pair targets disjoint tiles and can
                # co-issue in the PE array.
                for j in range(2):
                    mc = 2 * g + j
                    nc.tensor.matmul(
                        stA[:, j], lhsT=KT[0:64, c, mc * P : (mc + 1) * P],
                        rhs=q_a, start=True, stop=True, tile_position=(0, 0),
                    )
                    nc.tensor.matmul(
                        stB[:, j], lhsT=KT[64:128, c, mc * P : (mc + 1) * P],
                        rhs=q_b, start=True, stop=True, tile_position=(64, 0),
                    )
                pA = pp.tile([P, 2, 512], pdt, tag="p", name="pA")
                nc.scalar.activation(out=pA, in_=stA, func=AF.Exp, bias=esh_t)
                pB = pp.tile([P, 2, 512], pdt, tag="p", name="pB")
                nc.scalar.activation(out=pB, in_=stB, func=AF.Exp, bias=esh_t)
                if pend is not None:
                    emit_pv(*pend)
                pend = (g, pA, pB)
                pace(1)
            emit_pv(*pend)
            normalize(otA, hA, nt)
            normalize(otB, hB, nt)

        # ---- emission schedule -----------------------------------------
        # PE warmup: dummy matmuls fill the DMA-bound lead-in so the HAM
        # clock gate reaches 8/8 before the first real matmuls arrive.
        dummy = singles.tile([P, 512], bf16, name="dummy")
        nc.vector.memset(dummy, 0.5)
        for _ in range(24):
            wu = ps.tile([P, 512], f32, tag="bk", bufs=4, name="wu")
            nc.tensor.matmul(wu, lhsT=ident, rhs=dummy, start=True, stop=True)

        # upfront: everything attention(c=0, nt=0) groups 0..3 need; the
        # activation DMAs are emitted before the weight stages so the
        # input tiles win the DMA queues.
        for tg in (0, 1):
            for th in ln_group_thunks(csrc, DC, tg, gc, bc_, cnT):
                th()
        for th in ln_group_thunks(xsrc, DQ, 0, gx, bx, xnT):
            th()
        wq_b = load_weight(io["Wq"], DQ, "wq_b")
        wk_b = load_weight(io["Wk"], DC, "wk_b")
        wv_b = load_weight(io["Wv"], DC, "wv_b")
        k_proj(0, 0)
        k_proj(0, 1)
        q_proj(0, 0)
        for mc in range(4):
            v_proj(mc)
        wo_b = load_wo()

        # deferrable/background tasks with explicit producer deps; `order`
        # doubles as the pacing sequence (roughly consumption order).
        CL = 8  # last thunk index of a cond LN chain (9 thunks)
        for t in range(8):
            task(("xr", t), [], lambda t=t: xres_load(t))
        for tg in (2, 3):
            ths = ln_group_thunks(csrc, DC, tg, gc, bc_, cnT)
            prev = []
            for i, th in enumerate(ths):
                task(("c", tg, i), prev, th)
                prev = [("c", tg, i)]
            task(("k", 0, tg), prev, lambda ng=tg: k_proj(0, ng))
            for mc in range(4 * tg - 4, 4 * tg):
                task(("v", mc), [("c", mc // 4, CL)], lambda mc=mc: v_proj(mc))
        for mc in (12, 13, 14, 15):
            task(("v", mc), [("c", 3, CL)], lambda mc=mc: v_proj(mc))
        ths = ln_group_thunks(xsrc, DQ, 1, gx, bx, xnT)
        prev = []
        for i, th in enumerate(ths):
            task(("x1", i), prev, th)
            prev = [("x1", i)]
        x1_last = prev
        for m in (1, 2, 3):
            task(("q", m, 0), [], lambda m=m: q_proj(m, 0))
            for ng in range(4):
                task(("k", m, ng), [("c", ng, CL)],
                     lambda m=m, ng=ng: k_proj(m, ng))
        for m in range(4):
            task(("q", m, 1), x1_last, lambda m=m: q_proj(m, 1))

        # Block order interleaves the two query tiles so Wo(nt=0) work is
        # available as late PE filler.
        blocks = [(0, 0), (0, 1), (0, 2), (1, 0), (0, 3), (1, 1), (1, 2),
                  (1, 3)]
        for nt, c in blocks:
            ensure(("q", c, nt))
            attention(c, nt)
            wo_flush()
            if (nt, c) == (0, 3):
                for t in range(4):
                    task(("wo0", t), [("xr", t)], lambda t=t: wo_mm(t))
        pace(len(order))
        wo_flush()
        for t in range(4, 8):
            ensure(("xr", t))
            wo_mm(t)
        wo_flush()


def _pin_act_table_set():
    """Make Exp and Ln resolve to the one table set that holds both
    (natural_log_exp_and_others) so the kernel never thrashes ACT table
    loads between exp-only and ln-only sets.  Only the selection metadata
    is filtered; set ids stay aligned with act_info.json."""
    import concourse.bacc as bacc
    import concourse.mybir as mybir

    if getattr(bacc, "_act_tables_pinned", False):
        return
    AF = mybir.ActivationFunctionType
    orig = bacc.get_activation_tables

    def patched(arch):
        tables = dict(orig(arch))
        for name, fns in tables.items():
            if name != "natural_log_exp_and_others":
                fns.discard(AF.Exp)
                fns.discard(AF.Ln)
        return tables

    bacc.get_activation_tables = patched
    bacc._act_tables_pinned = True


@functools.cache
def _build_program():
    import concourse.bacc as bacc
    import concourse.mybir as mybir
    import concourse.tile as tile

    _pin_act_table_set()

    f32 = mybir.dt.float32
    nc = bacc.Bacc()
    io = {}
    io["x"] = nc.declare_dram_parameter("x", [NQ, DQ], f32, False)[:, :]
    io["cond"] = nc.declare_dram_parameter("cond", [M, DC], f32, False)[:, :]
    for name in ("lnx_g", "lnx_b"):
        io[name] = nc.declare_dram_parameter(name, [DQ], f32, False)[:]
    for name in ("lnc_g", "lnc_b"):
        io[name] = nc.declare_dram_parameter(name, [DC], f32, False)[:]
    io["Wq"] = nc.declare_dram_parameter("Wq", [DQ, INNER], f32, False)[:, :]
    io["Wk"] = nc.declare_dram_parameter("Wk", [DC, INNER], f32, False)[:, :]
    io["Wv"] = nc.declare_dram_parameter("Wv", [DC, INNER], f32, False)[:, :]
    io["Wo"] = nc.declare_dram_parameter("Wo", [INNER, DQ], f32, False)[:, :]
    for name in ("bo", "lnf_g", "lnf_b"):
        io[name] = nc.declare_dram_parameter(name, [DQ], f32, False)[:]
    io["out"] = nc.declare_dram_parameter("out", [NQ, DQ], f32, True)[:, :]

    with tile.TileContext(nc) as tc:
        _emit(tc, io)
    nc.compile()
    return nc


def _core_input_map(inputs, core):
    b, half = core // 2, core % 2
    m = {
        "x": np.ascontiguousarray(inputs["x"][b, half * NQ : (half + 1) * NQ]),
        "cond": np.ascontiguousarray(inputs["cond"][b]),
    }
    for name in (
        "lnx_g",
        "lnx_b",
        "lnc_g",
        "lnc_b",
        "Wq",
        "Wk",
        "Wv",
        "Wo",
        "bo",
        "lnf_g",
        "lnf_b",
    ):
        m[name] = np.asarray(inputs[name], dtype=np.float32)
    return m


TRACE = False
LAST_RESULTS = None


def kernel(**inputs):
    from concourse.bass_utils import run_bass_kernel_spmd

    global LAST_RESULTS
    nc = _build_program()
    in_maps = [_core_input_map(inputs, core) for core in range(N_CORES)]
    res = run_bass_kernel_spmd(
        nc,
        in_maps,
        list(range(N_CORES)),
        trace=TRACE,
        trace_cores=[0] if TRACE else None,
    )
    LAST_RESULTS = res
    out = np.empty((B, N, DQ), np.float32)
    for core in range(N_CORES):
        b, half = core // 2, core % 2
        out[b, half * NQ : (half + 1) * NQ] = res.results[core]["out"]
    return out
